# revision 25
# baseline (speedup 1.0000x reference)
"""AttentionAggregator Trainium2 kernel, v2.

B=20000 nodes, K=10 neighbors, N=100000 embed rows, F=256, H=128.
Data-parallel over B across 8 NeuronCores (2500 nodes/core).

v2 changes vs baseline (214.6us):
  - xt gather buffers 2->4: the 4 per-chunk transposed gathers run
    concurrently on the 4 SWDGE queues instead of lock-stepping with
    compute (gpsimd descriptor-gen was co-critical with PE).
  - tanh batched per gather-PAIR via [128,1024] 2-bank PSUM tiles:
    6 ACT insts/chunk instead of 12 (each ACT has a ~293ns pipe cost).
  - G copy moved from ACT to DVE, batched per pair (2/chunk).
  - mask via 2 bank-local 249-col matmuls into a unified SA psum tile
    [128, 2, 512] that holds S slots, then is overwritten by the rA
    outputs, with the out-matmul target carved from its spare region:
    PSUM = proj/G/V pool (4 banks) + SA pool (4 banks) = 8 exactly.
  - per-quad DVE: one 4D reciprocal, one broadcast (stride-0)
    tensor_tensor for as4 (replaces 4 tensor_scalars), bf16 out.
  - software-pipelined quads across the whole core (global quad
    stream): PE never waits on exp/DVE (rA runs 1 quad behind, out
    2 behind).
  - idx DMA split + early ACT table preload to cut startup.
"""

import sys
import dataclasses

sys.path.insert(0, "/opt/trn_rl_repo")

import numpy as np
import ml_dtypes

import concourse.bass as bass
import concourse.bacc as bacc
import concourse.mybir as mybir
import concourse.tile as tile
from concourse.bass_utils import run_bass_kernel_spmd

BF16 = ml_dtypes.bfloat16

B, K, N, F, H = 20000, 10, 100000, 256, 128
NCORES = 8
B_CORE = B // NCORES                # 2500
CHUNK_NODES = 192
CHUNK_ROWS = CHUNK_NODES * K        # 1920
NCHUNK = 13
TAIL_NODES = 48
TAIL_ROWS = 512
PAD_NODES = NCHUNK * CHUNK_NODES + TAIL_NODES   # 2544
PAD_ROWS = NCHUNK * CHUNK_ROWS + TAIL_ROWS      # 25472
IDX_COLS = NCHUNK * (CHUNK_ROWS // 16) + TAIL_ROWS // 16  # 1592
GSIZES = (512, 512, 512, 384)
GSIZES_LAST = (512, 512, 512, 384, 512)   # chunk 12 absorbs the tail rows
GOFFS = (0, 512, 1024, 1536, 1920)
GROUP_NODES = 12
GROUP_ROWS = GROUP_NODES * K        # 120
QUADS = 4
TBL_ROWS = 25600
MASK_L = 50.0

_CACHED = {}


def _build_program():
    nc = bacc.Bacc(
        "TRN2",
        target_bir_lowering=False,
        debug=False,
        num_devices=NCORES,
        num_swdge_queues=4,
    )
    dt = mybir.dt
    f32, bf16, i16 = dt.float32, dt.bfloat16, dt.int16

    table = nc.dram_tensor("table", [TBL_ROWS, F], bf16, kind="ExternalInput")
    idxs = nc.dram_tensor("idxs", [128, IDX_COLS], i16, kind="ExternalInput")
    # packed consts: 8 weight mats | um | vm2 | bi
    wnames = ["w1a0", "w1a1", "w2a0", "w2a1", "w3a0", "w3a1", "w3b", "cmt"]
    WPACK = 8 * 128 + 128 + 249 + GROUP_NODES
    wpack = nc.dram_tensor("wpack", [128, WPACK], bf16, kind="ExternalInput")
    out = nc.dram_tensor("out", [PAD_NODES, H], f32, kind="ExternalOutput")

    with tile.TileContext(nc) as tc:
        with (
            tc.tile_pool(name="consts", bufs=1) as cpool,
            tc.tile_pool(name="xt", bufs=4) as xpool,
            tc.tile_pool(name="tp", bufs=2) as tpool,
            tc.tile_pool(name="attn", bufs=2) as apool,
            tc.tile_pool(name="outst", bufs=2) as opool,
            tc.tile_pool(name="P", bufs=2, space="PSUM") as Ppool,
            tc.tile_pool(name="SAp", bufs=2, space="PSUM") as SApool,
        ):
            # ---- constants ----
            idx_sb = cpool.tile([128, IDX_COLS], i16, tag="c_idx")
            nc.sync.dma_start(out=idx_sb[:, 0:32], in_=idxs[:, 0:32])
            nc.sync.dma_start(out=idx_sb[:, 32:120], in_=idxs[:, 32:120])
            wp = cpool.tile([128, WPACK], bf16, tag="c_wpack")
            nc.sync.dma_start(out=wp[:, :], in_=wpack[:, :])
            wsb = {n: wp[:, 128 * i:128 * (i + 1)]
                   for i, n in enumerate(wnames)}
            um_sb = wp[0:13, 1024:1152]
            vm2_sb = wp[0:13, 1152:1401]
            bi_sb = wp[:, 1401:1401 + GROUP_NODES]
            # warm the ACT table set early (exp_and_others incl tanh)
            warm = cpool.tile([128, 8], bf16, tag="c_warm")
            nc.scalar.activation(warm[0:1, 0:1], wp[0:1, 0:1],
                                 mybir.ActivationFunctionType.Exp)
            nc.sync.dma_start(out=idx_sb[:, 120:], in_=idxs[:, 120:])

            PROJ = (("w1a0", "w1a1", 0), ("w2a0", "w2a1", 1),
                    ("w3a0", "w3a1", 2))

            def emit_gathers(c, sizes):
                xts = []
                for gi, gsz in enumerate(sizes):
                    xt = xpool.tile([128, 2, gsz], bf16, tag=f"xt{gi}")
                    nc.gpsimd.dma_gather(
                        out_ap=xt[:, :, :],
                        in_ap=table[:, :],
                        idxs_ap=idx_sb[:, c * 120 + GOFFS[gi] // 16:
                                       c * 120 + (GOFFS[gi] + gsz) // 16],
                        num_idxs=gsz,
                        num_idxs_reg=gsz,
                        elem_size=F,
                        transpose=True,
                        queue_num=gi % 4,
                    )
                    xts.append(xt)
                return xts

            def proj_units(xts, sizes, ts, g):
                """Closures for pair-batched proj+G units so deferred quad
                stages can interleave between them on the PE stream."""
                npair = (len(xts) + 1) // 2
                units = []
                for pi in range(npair):
                    ga = 2 * pi
                    sza = sizes[ga]
                    szb = sizes[ga + 1] if ga + 1 < len(xts) else 0
                    o = GOFFS[ga]
                    w = sza + szb

                    def junit(w0, w1, tj, ga=ga, sza=sza, szb=szb, o=o, w=w):
                        P = Ppool.tile([128, 1024], f32, tag="P", name="P")
                        nc.tensor.matmul(P[:, 0:sza], wsb[w0][:, :],
                                         xts[ga][:, 0, :],
                                         start=True, stop=False)
                        nc.tensor.matmul(P[:, 0:sza], wsb[w1][:, :],
                                         xts[ga][:, 1, :],
                                         start=False, stop=True)
                        if szb:
                            nc.tensor.matmul(P[:, 512:512 + szb],
                                             wsb[w0][:, :],
                                             xts[ga + 1][:, 0, :],
                                             start=True, stop=False)
                            nc.tensor.matmul(P[:, 512:512 + szb],
                                             wsb[w1][:, :],
                                             xts[ga + 1][:, 1, :],
                                             start=False, stop=True)
                        nc.scalar.activation(
                            ts[tj][:, o:o + w], P[:, 0:w],
                            mybir.ActivationFunctionType.Tanh)

                    def gunit(sza=sza, szb=szb, o=o, w=w):
                        Pg = Ppool.tile([128, 1024], f32, tag="P", name="Pg")
                        nc.tensor.matmul(Pg[:, 0:sza], wsb["cmt"][:, :],
                                         ts[0][:, o:o + sza],
                                         start=True, stop=True)
                        if szb:
                            nc.tensor.matmul(Pg[:, 512:512 + szb],
                                             wsb["cmt"][:, :],
                                             ts[0][:, o + sza:o + w],
                                             start=True, stop=True)
                        nc.vector.tensor_copy(g[:, o:o + w], Pg[:, 0:w])

                    for w0, w1, tj in PROJ:
                        units.append(lambda w0=w0, w1=w1, tj=tj, f=junit:
                                     f(w0, w1, tj))
                    units.append(gunit)
                return units

            # ---- software-pipelined quad stream ----
            # stage S: mask+S MMs -> SA slots; exp -> em
            # stage V: V MMs -> Pv; memset+cast -> v4
            # stage rA (1 behind): rA MMs overwrite SA slots
            # stage norm: DVE recip + broadcast as4 (bf16)
            # stage out (2 behind): out MMs -> spare of newer SA; copy outst
            def sa4_view(SA):
                return SA[0:120, :, 0:258].rearrange(
                    "p h (g c) -> p h g c", g=2)

            def stage_S(st):
                SA, t2, g = st["SA"], st["t2"], st["g"]
                rb = st["row_base"]
                for h in range(2):
                    nc.tensor.matmul(SA[:, h, 0:249], um_sb[:, :],
                                     vm2_sb[:, :], start=True, stop=False,
                                     skip_group_check=True)
                for qq in range(4):
                    h, gq = qq // 2, qq % 2
                    r0 = rb + GROUP_ROWS * qq
                    nc.tensor.matmul(
                        SA[:, h, 129 * gq:129 * gq + 120],
                        t2[:, r0:r0 + 128], g[:, r0:r0 + 120],
                        start=False, stop=True, skip_group_check=True)

            def stage_exp(st):
                em = st["em"]
                em4 = em[0:120, 0:480].rearrange(
                    "p (h g c) -> p h g c", h=2, g=2)
                nc.scalar.activation(em4, sa4_view(st["SA"])[:, :, :, 0:120],
                                     mybir.ActivationFunctionType.Exp)

            def stage_V_mm(st):
                t3, rb = st["t3"], st["row_base"]
                Pv = Ppool.tile([128, 1024], f32, tag="P", name="Pv")
                st["Pv"] = Pv
                for qq in range(4):
                    r0 = rb + GROUP_ROWS * qq
                    nc.tensor.matmul(Pv[:, 128 * qq:128 * (qq + 1)],
                                     t3[:, r0:r0 + 128], wsb["w3b"][:, :],
                                     start=True, stop=True)

            def stage_V_dve(st):
                Pv = st["Pv"]
                v4 = apool.tile([128, 4, 129], bf16, tag="v4")
                st["v4"] = v4
                nc.vector.memset(v4[0:120, :, 0:1], 1.0)
                nc.vector.tensor_copy(
                    v4[0:120, :, 1:129],
                    Pv[0:120, 0:512].rearrange("p (a b) -> p a b", a=4))

            def stage_rA(st):
                SA, em, v4 = st["SA"], st["em"], st["v4"]
                for qq in range(4):
                    h, gq = qq // 2, qq % 2
                    nc.tensor.matmul(SA[:, h, 129 * gq:129 * gq + 129],
                                     em[0:120, 120 * qq:120 * qq + 128],
                                     v4[0:120, qq, :],
                                     start=True, stop=True,
                                     skip_group_check=True)

            def stage_norm(st):
                SA = st["SA"]
                rec = apool.tile([128, 4], f32, tag="rec")
                st["rec"] = rec
                rec4 = rec[0:120, 0:4].rearrange(
                    "p (h g o) -> p h g o", h=2, g=2)
                sv = sa4_view(SA)
                nc.vector.reciprocal(rec4, sv[:, :, :, 0:1])
                as4 = apool.tile([128, 512], bf16, tag="as4")
                st["as4"] = as4
                as44 = as4[0:120, :].rearrange(
                    "p (h g c) -> p h g c", h=2, g=2)
                rb3 = rec[0:120, 0:4].rearrange("p (h g) -> p h g", h=2)
                rb4 = dataclasses.replace(rb3, ap=rb3.ap + [[0, 128]])
                nc.vector.tensor_tensor(out=as44, in0=sv[:, :, :, 1:129],
                                        in1=rb4, op=mybir.AluOpType.mult)

            def stage_out(st, SAdest):
                as4 = st["as4"]
                for qq in range(4):
                    nc.tensor.matmul(
                        SAdest[32 * qq:32 * qq + GROUP_NODES, 0, 258:386],
                        bi_sb[0:120, :],
                        as4[0:120, 128 * qq:128 * (qq + 1)],
                        start=True, stop=True, skip_group_check=True,
                        tile_position=(0, 32 * qq))
                # ACT does this copy: the DVE queue is busy enough that
                # routing it there delays the SA-freeing reads
                nc.scalar.activation(st["outst"][:, st["q"], :],
                                     SAdest[:, 0, 258:386],
                                     mybir.ActivationFunctionType.Copy)
                if st["q"] == st["qlast"]:
                    emit_store(st["c"], st["outst"])

            def make_quad(t2, t3, g, row_base, outst, q, c, qlast):
                SA = SApool.tile([128, 2, 512], f32, tag="SA")
                em = apool.tile([128, 488], bf16, tag="em")
                return {"SA": SA, "em": em, "t2": t2, "t3": t3, "g": g,
                        "row_base": row_base, "outst": outst, "q": q,
                        "c": c, "qlast": qlast}

            def emit_store(c, outst):
                base = c * CHUNK_NODES
                nq = 4 if c < NCHUNK else 1
                for qq in range(4):
                    dst = out[base + nq * GROUP_NODES * qq:
                              base + nq * GROUP_NODES * (qq + 1), :]
                    if nq > 1:
                        dst = dst.rearrange("(q i) d -> i q d",
                                            i=GROUP_NODES)
                        src = outst[32 * qq:32 * qq + GROUP_NODES, 0:nq, :]
                    else:
                        src = outst[32 * qq:32 * qq + GROUP_NODES, 0, :]
                    nc.sync.dma_start(out=dst, in_=src)

            pend = []          # quad pipeline, newest last (up to 2 states)

            def pump(newst):
                """PE order per pump t: V(t), mask+S(t), rA(t-1), out(t-2).
                Gives exp(t-1) a full pump before rA(t-1), and as4(t-2) a
                full pump before out(t-2), so the PE never idles long
                enough for the HAM to re-throttle."""
                stage_V_mm(newst)
                stage_S(newst)
                stage_exp(newst)
                if pend:
                    stage_rA(pend[-1])
                    stage_norm(pend[-1])
                stage_V_dve(newst)
                if len(pend) >= 2:
                    old = pend.pop(0)
                    stage_out(old, pend[-1]["SA"])
                pend.append(newst)

            # warm-up matmuls: keep the PE busy from weight-load until the
            # first gather lands so the HAM clock gate stays open and the
            # first projections run at full clock
            warmp = Ppool.tile([128, 1024], f32, tag="P", name="warmp")
            for _ in range(72):
                nc.tensor.matmul(warmp[:, 0:512], wsb["cmt"][:, :],
                                 wp[:, 0:512], start=True, stop=True)

            for c in range(NCHUNK):
                islast = c == NCHUNK - 1
                sizes = GSIZES_LAST if islast else GSIZES
                rows = 2440 if islast else CHUNK_ROWS + 8
                xts = emit_gathers(c, sizes)
                t1 = tpool.tile([128, rows], bf16, tag="t1",
                                padded_shape=[128, 2440])
                t2 = tpool.tile([128, rows], bf16, tag="t2",
                                padded_shape=[128, 2440])
                t3 = tpool.tile([128, rows], bf16, tag="t3",
                                padded_shape=[128, 2440])
                g = tpool.tile([128, rows - 8], bf16, tag="g",
                               padded_shape=[128, 2432])
                units = proj_units(xts, sizes, (t1, t2, t3), g)
                outst = opool.tile([128, 4, H], f32, tag="outst")
                nq = 5 if islast else 4
                quads = [None] * nq
                quads[0] = make_quad(t2, t3, g, 0, outst, 0, c, 3)
                # the previous chunk's trailing quad stages (rA of its last
                # quad, out of its last two) interleave into the proj units
                # so those PE stalls overlap the tanh chain instead
                fillers = []
                if pend:
                    prev = pend[-1]

                    def f_ra(prev=prev):
                        stage_rA(prev)
                        stage_norm(prev)
                    fillers.append(f_ra)
                    if len(pend) >= 2:
                        old2 = pend[0]

                        def f_out2(old2=old2, prev=prev):
                            stage_out(old2, prev["SA"])
                        fillers.append(f_out2)

                    def f_out1(prev=prev, dest=quads[0]["SA"]):
                        stage_out(prev, dest)
                    fillers.append(f_out1)
                pend.clear()

                def mkpump(q, t2=t2, t3=t3, g=g, outst=outst, c=c):
                    if quads[q] is None:
                        if q == 4:
                            # the absorbed tail quad stores as "chunk 13"
                            o13 = opool.tile([128, 4, H], f32, tag="outst")
                            quads[q] = make_quad(t2, t3, g, 480 * q, o13,
                                                 0, NCHUNK, 0)
                        else:
                            quads[q] = make_quad(t2, t3, g, 480 * q, outst,
                                                 q, c, 3)
                    pump(quads[q])

                # schedule: proj units interleaved with leftover stages and
                # the earliest pumps whose dependencies are met, so the PE
                # always has independent work while the tanh chain runs
                sched = [units[0], units[1]]
                if fillers:
                    sched.append(fillers[0])
                sched += [units[2]]
                if len(fillers) > 1:
                    sched.append(fillers[1])
                sched += [units[3]]
                sched += fillers[2:3]
                sched += [units[4], lambda: mkpump(0), units[5],
                          units[7], lambda: mkpump(1), units[6]]
                if islast:
                    sched += [units[8], lambda: mkpump(2), units[9],
                              units[10], lambda: mkpump(3), units[11],
                              lambda: mkpump(4)]
                else:
                    sched += [lambda: mkpump(2), lambda: mkpump(3)]
                for s in sched:
                    s()
            # drain: rA/out of the last two quads
            prev = pend[-1]
            stage_rA(prev)
            stage_norm(prev)
            if len(pend) >= 2:
                stage_out(pend[0], prev["SA"])
            SAfin = SApool.tile([128, 2, 512], f32, tag="SA")
            stage_out(prev, SAfin)

    nc.finalize()
    return nc


def _host_prep(neighbors, embed_table, W1a, W1b, W2a, W2b, W3a, W3b):
    """Shard + build per-core input maps."""
    embed_table = np.asarray(embed_table)
    ebf = np.ascontiguousarray(embed_table.astype(BF16))

    def b(x):
        return np.ascontiguousarray(np.asarray(x).astype(BF16))

    w1a, w2a, w3a = (np.asarray(w, np.float32) for w in (W1a, W2a, W3a))
    wmats = [
        w1a[0:128], w1a[128:256], w2a[0:128], w2a[128:256],
        w3a[0:128], w3a[128:256], np.asarray(W3b, np.float32),
        np.asarray(W1b, np.float32) @ np.asarray(W2b, np.float32).T,
    ]
    # mask = um^T @ vm2 adds 0 in-block, -L off-block (rank 13);
    # vm2 covers 2 group slots [0:120 | gap 9 | 129:249] per psum bank
    bi = np.zeros((120, GROUP_NODES), np.float32)
    for p in range(120):
        bi[p, p // K] = 1.0
    um = np.zeros((128, 128), np.float32)
    um[0:12, 0:120] = bi.T
    um[12, 0:120] = 1.0
    vm2 = np.zeros((128, 249), np.float32)
    for off in (0, 129):
        vm2[0:12, off:off + 120] = MASK_L * bi.T
        vm2[12, off:off + 120] = -MASK_L
    bi128 = np.zeros((128, GROUP_NODES), np.float32)
    bi128[0:120] = bi
    wpack = np.concatenate(wmats + [um, vm2, bi128], axis=1)
    shared = {"wpack": b(wpack)}

    nbr = np.asarray(neighbors).astype(np.int64)
    in_maps = []
    for c in range(NCORES):
        nb_c = nbr[c * B_CORE:(c + 1) * B_CORE]           # [2500, 10]
        uniq, inv = np.unique(nb_c, return_inverse=True)
        assert uniq.size <= TBL_ROWS
        tbl = np.zeros((TBL_ROWS, F), BF16)
        tbl[:uniq.size] = ebf[uniq]
        # permute nodes within full chunks so the packed-psum output slabs
        # land on contiguous out rows: slot 12*(4q+qq)+j <- node 48qq+12q+j
        perm = np.empty(CHUNK_NODES, np.int64)
        for pq in range(4):
            for pqq in range(4):
                for pj in range(GROUP_NODES):
                    perm[GROUP_NODES * (4 * pq + pqq) + pj] = \
                        48 * pqq + GROUP_NODES * pq + pj
        nodes = np.zeros((PAD_NODES, K), np.int16)
        nodes[:B_CORE] = inv.astype(np.int16).reshape(B_CORE, K)
        for ch in range(NCHUNK):
            blk = nodes[ch * CHUNK_NODES:(ch + 1) * CHUNK_NODES].copy()
            nodes[ch * CHUNK_NODES:(ch + 1) * CHUNK_NODES] = blk[perm]
        flat = np.zeros(PAD_ROWS, np.int16)
        flat[:nodes.size] = nodes.ravel()
        # wrap: index j of a chunk at [j % 16, j // 16], replicated to 128
        idx128 = np.zeros((128, IDX_COLS), np.int16)
        col = row = 0
        for sz in [CHUNK_ROWS] * NCHUNK + [TAIL_ROWS]:
            blk = flat[row:row + sz].reshape(sz // 16, 16).T
            idx128[:, col:col + sz // 16] = np.tile(blk, (8, 1))
            row += sz
            col += sz // 16
        in_maps.append({
            "table": tbl,
            "idxs": idx128,
            **{k: v for k, v in shared.items()},
        })
    return in_maps


def kernel(neighbors, embed_table, W1a, W1b, W2a, W2b, W3a, W3b, _trace=False,
           **trace_kwargs):
    key = "prog"
    if key not in _CACHED:
        _CACHED[key] = _build_program()
    nc = _CACHED[key]
    in_maps = _host_prep(neighbors, embed_table, W1a, W1b, W2a, W2b, W3a, W3b)
    res = None
    for attempt in range(3):
        try:
            res = run_bass_kernel_spmd(nc, in_maps, list(range(NCORES)),
                                       trace=_trace, **trace_kwargs)
            break
        except Exception:
            # the axon/TRN2 device occasionally wedges transiently; retry
            if attempt == 2:
                raise
            import time
            time.sleep(5)
    outs = [res.results[c]["out"][:B_CORE] for c in range(NCORES)]
    full = np.concatenate(outs, axis=0).astype(np.float32)
    kernel.last_results = res
    return full
